# revision 19
# baseline (speedup 1.0000x reference)
"""Trainium2 Bass kernel for nn_ActorFlowODE (dense MLP flow ODE actor).

Data-parallel over 8 NeuronCores: batch 32768 -> 4096 rows/core, weights
replicated. Feature-major activations on-chip; f32r (reduced fp32) matmuls;
mish via exact Exp/Square/Ln/Exp LUT chain; LayerNorm folded algebraically
into the following layer's weights (host precompute) with per-batch mean
applied through a K=1 fixup matmul and rsigma through a broadcast multiply.
The obs @ W0[:512] product (+ b0) is tau/z-independent, so it is computed
once per forward and streamed from DRAM for each of the 4 velocity evals.
"""

import numpy as np

import concourse.bass as bass
import concourse.tile as tile
from concourse import mybir
from concourse.bass_utils import run_bass_kernel_spmd
from concourse.masks import make_identity

F32 = mybir.dt.float32
F32R = mybir.dt.float32r
AF = mybir.ActivationFunctionType
ALU = mybir.AluOpType

N_CORES = 8
OBS_DIM, ACT_DIM = 512, 64
H = 1024
LN_EPS = 1e-5
MIN_LOGSTD = -10.0
DT = 0.5  # 1 / K_SUBSTEPS
NB = 512  # batch-chunk (moving free dim)
MC = H // 128  # 8 feature chunks of the hidden layer
TAUS = (0.0, 0.5, 0.5, 1.0)  # tau for evals (k1, k2, k1, k2)


# ---------------------------------------------------------------------------
# Workaround: walrus in this container accepts at most ONE sync wait per
# instruction. Split any instruction carrying N>1 waits into N-1 single-wait
# NoOps on the same engine placed just before it.
_uid = [0]


def _split_multi_waits(nc):
    for f in nc.m.functions:
        for bb in f.blocks:
            insts = bb.instructions
            new = []
            changed = False
            for inst in insts:
                si = inst.sync_info
                waits = list(si.on_wait) if si is not None else []
                if len(waits) > 1:
                    changed = True
                    for w in waits[:-1]:
                        _uid[0] += 1
                        nop = mybir.InstNoOp(
                            name=f"I-waitsplit-{_uid[0]}", ins=[], outs=[]
                        )
                        nop.engine = inst.engine
                        nop.sync_info = mybir.SyncInfo(on_wait=[w], on_update=[])
                        new.append(nop)
                    inst.sync_info = mybir.SyncInfo(
                        on_wait=[waits[-1]], on_update=list(si.on_update)
                    )
                new.append(inst)
            if changed:
                bb.instructions = new


# ---------------------------------------------------------------------------


def build_graph(n_bc):
    """Build the per-core Bass graph. n_bc = number of 512-row batch chunks
    per core (8 for the full problem)."""
    B = n_bc * NB
    nc = bass.Bass("TRN2", target_bir_lowering=False, debug=False,
                   num_devices=N_CORES)

    # -------- DRAM parameters (per-core shards / replicated weights) -------
    obs_e = nc.declare_dram_parameter("obs", [B, OBS_DIM], F32, isOutput=False)
    eps_e = nc.declare_dram_parameter("eps", [B, ACT_DIM], F32, isOutput=False)
    w0a_e = nc.declare_dram_parameter("w0a", [OBS_DIM, H], F32, isOutput=False)
    w0zx_e = nc.declare_dram_parameter("w0zx", [128, H], F32, isOutput=False)
    b0c_e = nc.declare_dram_parameter("b0c", [128, MC], F32, isOutput=False)
    w1p_e = nc.declare_dram_parameter("w1p", [H, H], F32, isOutput=False)
    negc1_e = nc.declare_dram_parameter("negc1", [1, H], F32, isOutput=False)
    d1c_e = nc.declare_dram_parameter("d1c", [128, MC], F32, isOutput=False)
    w2p_e = nc.declare_dram_parameter("w2p", [H, ACT_DIM], F32, isOutput=False)
    negc2_e = nc.declare_dram_parameter("negc2", [1, ACT_DIM], F32, isOutput=False)
    dtd2_e = nc.declare_dram_parameter("dtd2", [ACT_DIM, 1], F32, isOutput=False)
    std_e = nc.declare_dram_parameter("std", [ACT_DIM, 1], F32, isOutput=False)
    out_e = nc.declare_dram_parameter("out", [B, ACT_DIM], F32, isOutput=True)

    # P' = obs @ W0a + b0, feature-major, stored f32r in DRAM between evals
    p0_d = nc.dram_tensor("p0", [MC, 128, B], F32R)

    with tile.TileContext(nc) as tc:
        with (
            tc.tile_pool(name="const", bufs=1) as const,
            tc.tile_pool(name="acts", bufs=3) as acts,
            tc.tile_pool(name="tmp", bufs=4) as tmp,
            tc.tile_pool(name="tmp2", bufs=3) as tmp2,
            tc.tile_pool(name="p0pool", bufs=2) as p0pool,
            tc.tile_pool(name="msqp", bufs=4) as msqp,
            tc.tile_pool(name="statf", bufs=3) as statf,
            tc.tile_pool(name="statr", bufs=2) as statr,
            tc.tile_pool(name="abc", bufs=2) as abc,
            tc.tile_pool(name="stg", bufs=2) as stg,
            tc.tile_pool(name="outp", bufs=1) as outp,
            tc.tile_pool(name="pm", bufs=4, space="PSUM") as pm,
            tc.tile_pool(name="pstat", bufs=2, space="PSUM") as pstat,
            tc.tile_pool(name="pstatq", bufs=1, space="PSUM") as pstatq,
            tc.tile_pool(name="pb", bufs=1, space="PSUM") as pb,
        ):
            # ---------------- constants -----------------------------------
            ident = const.tile([128, 128], F32)
            make_identity(nc, ident[:])
            ones_f = const.tile([128, 1], F32)
            nc.vector.memset(ones_f[:], 1.0)
            ones = const.tile([128, 1], F32R)
            nc.scalar.copy(ones[:], ones_f[:])
            onesrow_f = const.tile([1, 128], F32)
            nc.vector.memset(onesrow_f[:], 1.0)
            onesrow = const.tile([1, 128], F32R)
            nc.scalar.copy(onesrow[:], onesrow_f[:])

            w0zx = const.tile([128, H], F32R)
            nc.gpsimd.dma_start(w0zx[:], w0zx_e.ap())
            w1p = const.tile([128, MC, H], F32R)
            nc.gpsimd.dma_start(
                w1p[:], w1p_e.ap().rearrange("(ko ki) m -> ki ko m", ki=128)
            )
            w2p = const.tile([128, MC, ACT_DIM], F32R)
            nc.gpsimd.dma_start(
                w2p[:], w2p_e.ap().rearrange("(ko ki) m -> ki ko m", ki=128)
            )
            negc1 = const.tile([1, H], F32R)
            nc.gpsimd.dma_start(negc1[:], negc1_e.ap())
            negc2 = const.tile([1, ACT_DIM], F32R)
            nc.gpsimd.dma_start(negc2[:], negc2_e.ap())
            b0c = const.tile([128, MC], F32)
            nc.sync.dma_start(b0c[:], b0c_e.ap())
            d1c = const.tile([128, MC], F32)
            nc.sync.dma_start(d1c[:], d1c_e.ap())
            dtd2 = const.tile([ACT_DIM, 1], F32)
            nc.sync.dma_start(dtd2[:], dtd2_e.ap())
            stdv = const.tile([ACT_DIM, 1], F32)
            nc.sync.dma_start(stdv[:], std_e.ap())

            # persistent state (feature-major). zx ping-pong: evals 0,2 read
            # zxa; evals 1,3 read zxb (rows 0:64 z / z_pred, row 64 tau).
            zxa = const.tile([128, B], F32R)
            zxb = const.tile([128, B], F32R)
            z = const.tile([ACT_DIM, B], F32)
            k1dt = const.tile([ACT_DIM, B], F32)
            tausrc = const.tile([1, NB], F32)

            # zero zx rows 64..127 (write via ACT so the f32r round is legal)
            zsrc = const.tile([64, NB], F32)
            nc.vector.memset(zsrc[:], 0.0)
            for s in range(n_bc):
                nc.scalar.copy(zxa[64:128, s * NB:(s + 1) * NB], zsrc[:])
                nc.scalar.copy(zxb[64:128, s * NB:(s + 1) * NB], zsrc[:])

            # ---------------- eps -> z0 (transpose + scale by std) ---------
            for bb in range(B // 128):
                stage = stg.tile([128, 128], F32, tag="stg")
                nc.sync.dma_start(stage[:, :ACT_DIM],
                                  eps_e[bb * 128:(bb + 1) * 128, :])
                pt = pb.tile([128, 512], F32, tag="pbt")
                nc.tensor.transpose(pt[:ACT_DIM, :128], stage[:, :ACT_DIM],
                                    ident[:])
                sl = slice(bb * 128, (bb + 1) * 128)
                nc.scalar.activation(z[:, sl], pt[:ACT_DIM, :128],
                                     AF.Identity, scale=stdv[:])
                nc.scalar.activation(zxa[0:ACT_DIM, sl], pt[:ACT_DIM, :128],
                                     AF.Identity, scale=stdv[:])

            # ---------------- GEMM0: P' = obs @ W0a + b0 -------------------
            w0a = acts.tile([128, 4, H], F32R, tag="acts")
            nc.gpsimd.dma_start(
                w0a[:], w0a_e.ap().rearrange("(ko ki) m -> ki ko m", ki=128)
            )
            for bc in range(n_bc):
                obst = acts.tile([128, 4, NB], F32R, tag="acts")
                for fb in range(4):
                    for sub in range(4):
                        stage = stg.tile([128, 128], F32, tag="stg")
                        nc.sync.dma_start(
                            stage[:],
                            obs_e[(bc * 4 + sub) * 128:(bc * 4 + sub + 1) * 128,
                                  fb * 128:(fb + 1) * 128],
                        )
                        pt = pb.tile([128, 512], F32, tag="pbt")
                        nc.tensor.transpose(pt[:, :128], stage[:], ident[:])
                        nc.scalar.copy(
                            obst[:, fb, sub * 128:(sub + 1) * 128], pt[:, :128]
                        )
                for mc in range(MC):
                    pp = pm.tile([128, NB], F32, tag="pm")
                    for fb in range(4):
                        nc.tensor.matmul(
                            pp[:], w0a[:, fb, mc * 128:(mc + 1) * 128],
                            obst[:, fb, :], start=(fb == 0), stop=(fb == 3),
                        )
                    dr = p0pool.tile([128, NB], F32R, tag="p0")
                    nc.scalar.activation(dr[:], pp[:], AF.Identity,
                                         bias=b0c[:, mc:mc + 1])
                    nc.sync.dma_start(p0_d[mc, :, bc * NB:(bc + 1) * NB], dr[:])

            # ---------------- helper: LN stats chain -----------------------
            def stats_chain(sps, spq):
                """sps/spq: psum [1,NB] (S and Q). Returns (mu f32r,
                a f32r) with mu = S/F, a = 1/sqrt(Q/F - mu^2 + eps)."""
                mu = statr.tile([1, NB], F32R, tag="statr")
                nc.vector.tensor_scalar_mul(mu[:], sps[:], 1.0 / H)
                musq = statf.tile([1, NB], F32, tag="statf")
                nc.vector.tensor_mul(musq[:], mu[:], mu[:])
                qfe = statf.tile([1, NB], F32, tag="statf")
                nc.vector.tensor_scalar(qfe[:], spq[:], 1.0 / H, LN_EPS,
                                        ALU.mult, ALU.add)
                var = statf.tile([1, NB], F32, tag="statf")
                nc.vector.tensor_tensor(var[:], qfe[:], musq[:], ALU.subtract)
                lv = statf.tile([1, NB], F32, tag="statf")
                nc.scalar.activation(lv[:], var[:], AF.Ln)
                a = statr.tile([1, NB], F32R, tag="statr")
                nc.scalar.activation(a[:], lv[:], AF.Exp, scale=-0.5)
                return mu, a

            def bcast(a_row):
                """[1,NB] f32r -> [128,NB] f32 via K=1 matmul + DVE copy."""
                pbt = pb.tile([128, 512], F32, tag="pbt")
                nc.tensor.matmul(pbt[:, :NB], onesrow[:], a_row[:],
                                 start=True, stop=True)
                ab = abc.tile([128, NB], F32, tag="abc")
                nc.vector.tensor_copy(ab[:], pbt[:, :NB])
                return ab

            # ---------------- the 4 velocity evals -------------------------
            for e in range(4):
                tau = TAUS[e]
                is_k1 = (e % 2 == 0)
                zxr = zxa if e % 2 == 0 else zxb   # tile read by this eval
                zxw = zxb if e % 2 == 0 else zxa   # tile written (z_pred/z)
                # tau row of the tile this eval reads
                nc.vector.memset(tausrc[:], tau)
                for s in range(n_bc):
                    nc.scalar.copy(zxr[64:65, s * NB:(s + 1) * NB], tausrc[:])

                for bc in range(n_bc):
                    bsl = slice(bc * NB, (bc + 1) * NB)
                    # ---- L0 ----
                    m0t = acts.tile([128, MC, NB], F32R, tag="acts")
                    sps = pstat.tile([1, NB], F32, tag="sum")
                    spq = pstatq.tile([1, NB], F32, tag="ssq")
                    pend = []
                    for pr in range(MC // 2):
                        up = tmp2.tile([128, 2, NB], F32, tag="tmp2")
                        y0s = []
                        for j in range(2):
                            mc = pr * 2 + j
                            pp = pm.tile([128, NB], F32, tag="pm")
                            nc.tensor.matmul(
                                pp[:], w0zx[:, mc * 128:(mc + 1) * 128],
                                zxr[:, bsl], start=True, stop=True)
                            p0t = p0pool.tile([128, NB], F32, tag="p0")
                            nc.sync.dma_start(p0t[:],
                                              p0_d[mc, :, bsl].bitcast(F32))
                            y0t = tmp.tile([128, NB], F32, tag="tmp")
                            nc.vector.tensor_tensor(y0t[:], pp[:], p0t[:],
                                                    ALU.add)
                            nc.scalar.activation(up[:, j, :], y0t[:], AF.Exp)
                            y0s.append(y0t)
                        nc.scalar.activation(up[:], up[:], AF.Square, bias=1.0)
                        nc.scalar.activation(up[:], up[:], AF.Ln, bias=1.0)
                        nc.scalar.activation(up[:], up[:], AF.Exp, scale=-1.0)
                        nc.gpsimd.tensor_scalar(up[:], up[:], -2.0, 1.0,
                                                ALU.mult, ALU.add)
                        for j in range(2):
                            mc = pr * 2 + j
                            nc.vector.tensor_mul(m0t[:, mc, :], y0s[j][:],
                                                 up[:, j, :])
                            ms = msqp.tile([128, NB], F32R, tag="msq")
                            nc.gpsimd.tensor_tensor(ms[:], m0t[:, mc, :],
                                                    m0t[:, mc, :], ALU.mult)
                            pend.append((mc, ms))
                        while len(pend) > 2:
                            mc, ms = pend.pop(0)
                            nc.tensor.matmul(sps[:], ones[:],
                                             m0t[:, mc, :], start=(mc == 0),
                                             stop=(mc == MC - 1))
                            nc.tensor.matmul(spq[:], ones[:], ms[:],
                                             start=(mc == 0),
                                             stop=(mc == MC - 1))
                    for mc, ms in pend:
                        nc.tensor.matmul(sps[:], ones[:], m0t[:, mc, :],
                                         start=(mc == 0), stop=(mc == MC - 1))
                        nc.tensor.matmul(spq[:], ones[:], ms[:],
                                         start=(mc == 0), stop=(mc == MC - 1))
                    mu0, a0 = stats_chain(sps, spq)
                    a0b = bcast(a0)
                    # ---- L1 ----
                    m1t = acts.tile([128, MC, NB], F32R, tag="acts")
                    sps1 = pstat.tile([1, NB], F32, tag="sum")
                    spq1 = pstatq.tile([1, NB], F32, tag="ssq")
                    pend = []
                    for pr in range(MC // 2):
                        up = tmp2.tile([128, 2, NB], F32, tag="tmp2")
                        y1s = []
                        for j in range(2):
                            mc = pr * 2 + j
                            pp = pm.tile([128, NB], F32, tag="pm")
                            for kc in range(MC):
                                nc.tensor.matmul(
                                    pp[:], w1p[:, kc, mc * 128:(mc + 1) * 128],
                                    m0t[:, kc, :], start=(kc == 0), stop=False,
                                )
                            nc.tensor.matmul(
                                pp[:], negc1[:, mc * 128:(mc + 1) * 128],
                                mu0[:], start=False, stop=True)
                            t = tmp.tile([128, NB], F32, tag="tmp")
                            nc.vector.tensor_mul(t[:], pp[:], a0b[:])
                            nc.vector.tensor_scalar_add(t[:], t[:],
                                                        d1c[:, mc:mc + 1])
                            nc.scalar.activation(up[:, j, :], t[:], AF.Exp)
                            y1s.append(t)
                        nc.scalar.activation(up[:], up[:], AF.Square, bias=1.0)
                        nc.scalar.activation(up[:], up[:], AF.Ln, bias=1.0)
                        nc.scalar.activation(up[:], up[:], AF.Exp, scale=-1.0)
                        nc.gpsimd.tensor_scalar(up[:], up[:], -2.0, 1.0,
                                                ALU.mult, ALU.add)
                        for j in range(2):
                            mc = pr * 2 + j
                            nc.vector.tensor_mul(m1t[:, mc, :], y1s[j][:],
                                                 up[:, j, :])
                            ms = msqp.tile([128, NB], F32R, tag="msq")
                            nc.vector.tensor_mul(ms[:], m1t[:, mc, :],
                                                 m1t[:, mc, :])
                            pend.append((mc, ms))
                        while len(pend) > 2:
                            mc, ms = pend.pop(0)
                            nc.tensor.matmul(sps1[:], ones[:],
                                             m1t[:, mc, :], start=(mc == 0),
                                             stop=(mc == MC - 1))
                            nc.tensor.matmul(spq1[:], ones[:], ms[:],
                                             start=(mc == 0),
                                             stop=(mc == MC - 1))
                    for mc, ms in pend:
                        nc.tensor.matmul(sps1[:], ones[:], m1t[:, mc, :],
                                         start=(mc == 0), stop=(mc == MC - 1))
                        nc.tensor.matmul(spq1[:], ones[:], ms[:],
                                         start=(mc == 0), stop=(mc == MC - 1))
                    mu1, a1 = stats_chain(sps1, spq1)
                    a1b = bcast(a1)
                    # ---- L2 (output head) ----
                    pv = pm.tile([128, NB], F32, tag="pm")
                    for kc in range(MC):
                        nc.tensor.matmul(pv[:ACT_DIM, :], w2p[:, kc, :],
                                         m1t[:, kc, :], start=(kc == 0),
                                         stop=False)
                    nc.tensor.matmul(pv[:ACT_DIM, :], negc2[:], mu1[:],
                                     start=False, stop=True)
                    t2 = tmp.tile([128, NB], F32, tag="tmp")
                    nc.vector.tensor_mul(t2[:ACT_DIM], pv[:ACT_DIM, :],
                                         a1b[:ACT_DIM])
                    # dk = dt*(v + d2) = dt*t2 + dt*d2
                    if is_k1:
                        nc.scalar.activation(k1dt[:, bsl], t2[:ACT_DIM],
                                             AF.Identity, bias=dtd2[:],
                                             scale=DT)
                        # z_pred into the other zx tile's rows 0:64
                        nc.vector.tensor_tensor(zxw[0:ACT_DIM, bsl], z[:, bsl],
                                                k1dt[:, bsl], ALU.add)
                    else:
                        dk = tmp.tile([128, NB], F32, tag="tmp")
                        nc.scalar.activation(dk[:ACT_DIM], t2[:ACT_DIM],
                                             AF.Identity, bias=dtd2[:],
                                             scale=DT)
                        s = tmp.tile([128, NB], F32, tag="tmp")
                        nc.vector.tensor_tensor(s[:ACT_DIM], k1dt[:, bsl],
                                                dk[:ACT_DIM], ALU.add)
                        h = tmp.tile([128, NB], F32, tag="tmp")
                        nc.vector.tensor_scalar_mul(h[:ACT_DIM], s[:ACT_DIM],
                                                    0.5)
                        nc.vector.tensor_tensor(z[:, bsl], z[:, bsl],
                                                h[:ACT_DIM], ALU.add)
                        if e == 1:
                            nc.scalar.copy(zxw[0:ACT_DIM, bsl], z[:, bsl])

            # ---------------- output: z^T -> out [B, 64] -------------------
            for bb in range(B // 128):
                pt = pb.tile([128, 512], F32, tag="pbt")
                nc.tensor.transpose(pt[:, :ACT_DIM],
                                    z[:, bb * 128:(bb + 1) * 128],
                                    ident[:ACT_DIM, :ACT_DIM])
                ot = outp.tile([128, ACT_DIM], F32, tag="out")
                nc.scalar.copy(ot[:], pt[:, :ACT_DIM])
                nc.sync.dma_start(out_e[bb * 128:(bb + 1) * 128, :], ot[:])

    _split_multi_waits(nc)
    return nc


# ---------------------------------------------------------------------------


def _host_params(inputs):
    obs = np.asarray(inputs["obs"], dtype=np.float32)
    eps = np.asarray(inputs["eps"], dtype=np.float32)
    logstd = np.asarray(inputs["logstd"], dtype=np.float32)
    W0 = np.asarray(inputs["W0"], dtype=np.float32)
    b0 = np.asarray(inputs["b0"], dtype=np.float32)
    g0 = np.asarray(inputs["ln0_g"], dtype=np.float32)
    be0 = np.asarray(inputs["ln0_b"], dtype=np.float32)
    W1 = np.asarray(inputs["W1"], dtype=np.float32)
    b1 = np.asarray(inputs["b1"], dtype=np.float32)
    g1 = np.asarray(inputs["ln1_g"], dtype=np.float32)
    be1 = np.asarray(inputs["ln1_b"], dtype=np.float32)
    W2 = np.asarray(inputs["W2"], dtype=np.float32)
    b2 = np.asarray(inputs["b2"], dtype=np.float32)

    std = np.exp(np.clip(logstd, MIN_LOGSTD, None)).astype(np.float32)

    w0a = np.ascontiguousarray(W0[:OBS_DIM])                      # [512,1024]
    w0zx = np.zeros((128, H), dtype=np.float32)
    w0zx[:ACT_DIM] = W0[OBS_DIM:OBS_DIM + ACT_DIM]
    w0zx[ACT_DIM] = W0[OBS_DIM + ACT_DIM]                         # tau row
    b0c = np.ascontiguousarray(b0.reshape(MC, 128).T)             # [128,8]

    w1p = (g0[:, None] * W1).astype(np.float32)                   # [1024,1024]
    negc1 = np.ascontiguousarray(-w1p.sum(axis=0)[None, :])       # [1,1024]
    d1 = (be0 @ W1 + b1).astype(np.float32)
    d1c = np.ascontiguousarray(d1.reshape(MC, 128).T)             # [128,8]

    w2p = (g1[:, None] * W2).astype(np.float32)                   # [1024,64]
    negc2 = np.ascontiguousarray(-w2p.sum(axis=0)[None, :])       # [1,64]
    d2 = (be1 @ W2 + b2).astype(np.float32)
    dtd2 = np.ascontiguousarray((DT * d2)[:, None])               # [64,1]

    shared = {
        "w0a": w0a, "w0zx": w0zx, "b0c": b0c,
        "w1p": w1p, "negc1": negc1, "d1c": d1c,
        "w2p": w2p, "negc2": negc2, "dtd2": dtd2,
        "std": np.ascontiguousarray(std[:, None]),
    }
    return obs, eps, shared


_graph_cache = {}


def kernel(**inputs):
    obs, eps, shared = _host_params(inputs)
    B = obs.shape[0]
    assert B % N_CORES == 0
    bc_per = B // N_CORES
    assert bc_per % NB == 0
    n_bc = bc_per // NB

    if n_bc not in _graph_cache:
        _graph_cache[n_bc] = build_graph(n_bc)
    nc = _graph_cache[n_bc]

    in_maps = []
    for c in range(N_CORES):
        sl = slice(c * bc_per, (c + 1) * bc_per)
        m = {"obs": np.ascontiguousarray(obs[sl]),
             "eps": np.ascontiguousarray(eps[sl])}
        m.update(shared)
        in_maps.append(m)

    res = run_bass_kernel_spmd(nc, in_maps, core_ids=list(range(N_CORES)))
    out = np.concatenate([res.results[c]["out"] for c in range(N_CORES)],
                         axis=0)
    return out.astype(np.float32)


# revision 20
# speedup vs baseline: 1.2203x; 1.2203x over previous
"""Trainium2 Bass kernel for nn_ActorFlowODE (dense MLP flow ODE actor).

Data-parallel over 8 NeuronCores: batch 32768 -> 4096 rows/core, weights
replicated. Feature-major activations on-chip; f32r (reduced fp32) matmuls;
mish via exact Exp/Square/Ln/Exp LUT chain; LayerNorm folded algebraically
into the following layer's weights (host precompute) with per-batch mean
applied through a K=1 fixup matmul and rsigma through a broadcast multiply.
The obs @ W0[:512] product (+ b0) is tau/z-independent, so it is computed
once per forward and streamed from DRAM for each of the 4 velocity evals.
"""

import numpy as np

import concourse.bass as bass
import concourse.tile as tile
from concourse import mybir
from concourse.bass_utils import run_bass_kernel_spmd
from concourse.masks import make_identity

F32 = mybir.dt.float32
F32R = mybir.dt.float32r
AF = mybir.ActivationFunctionType
ALU = mybir.AluOpType

N_CORES = 8
OBS_DIM, ACT_DIM = 512, 64
H = 1024
LN_EPS = 1e-5
MIN_LOGSTD = -10.0
DT = 0.5  # 1 / K_SUBSTEPS
NB = 512  # batch-chunk (moving free dim)
MC = H // 128  # 8 feature chunks of the hidden layer
TAUS = (0.0, 0.5, 0.5, 1.0)  # tau for evals (k1, k2, k1, k2)


# ---------------------------------------------------------------------------
# Workaround: walrus in this container accepts at most ONE sync wait per
# instruction. Split any instruction carrying N>1 waits into N-1 single-wait
# NoOps on the same engine placed just before it.
_uid = [0]


def _split_multi_waits(nc):
    for f in nc.m.functions:
        for bb in f.blocks:
            insts = bb.instructions
            new = []
            changed = False
            for inst in insts:
                si = inst.sync_info
                waits = list(si.on_wait) if si is not None else []
                if len(waits) > 1:
                    changed = True
                    for w in waits[:-1]:
                        _uid[0] += 1
                        nop = mybir.InstNoOp(
                            name=f"I-waitsplit-{_uid[0]}", ins=[], outs=[]
                        )
                        nop.engine = inst.engine
                        nop.sync_info = mybir.SyncInfo(on_wait=[w], on_update=[])
                        new.append(nop)
                    inst.sync_info = mybir.SyncInfo(
                        on_wait=[waits[-1]], on_update=list(si.on_update)
                    )
                new.append(inst)
            if changed:
                bb.instructions = new


# ---------------------------------------------------------------------------


def build_graph(n_bc):
    """Build the per-core Bass graph. n_bc = number of 512-row batch chunks
    per core (8 for the full problem)."""
    B = n_bc * NB
    nc = bass.Bass("TRN2", target_bir_lowering=False, debug=False,
                   num_devices=N_CORES)

    # -------- DRAM parameters (per-core shards / replicated weights) -------
    obs_e = nc.declare_dram_parameter("obs", [B, OBS_DIM], F32, isOutput=False)
    eps_e = nc.declare_dram_parameter("eps", [B, ACT_DIM], F32, isOutput=False)
    w0a_e = nc.declare_dram_parameter("w0a", [OBS_DIM, H], F32, isOutput=False)
    w0zx_e = nc.declare_dram_parameter("w0zx", [128, H], F32, isOutput=False)
    b0c_e = nc.declare_dram_parameter("b0c", [128, MC], F32, isOutput=False)
    w1p_e = nc.declare_dram_parameter("w1p", [H, H], F32, isOutput=False)
    negc1_e = nc.declare_dram_parameter("negc1", [1, H], F32, isOutput=False)
    d1c_e = nc.declare_dram_parameter("d1c", [128, MC], F32, isOutput=False)
    w2p_e = nc.declare_dram_parameter("w2p", [H, ACT_DIM], F32, isOutput=False)
    negc2_e = nc.declare_dram_parameter("negc2", [1, ACT_DIM], F32, isOutput=False)
    dtd2_e = nc.declare_dram_parameter("dtd2", [ACT_DIM, 1], F32, isOutput=False)
    std_e = nc.declare_dram_parameter("std", [ACT_DIM, 1], F32, isOutput=False)
    out_e = nc.declare_dram_parameter("out", [B, ACT_DIM], F32, isOutput=True)

    # P' = obs @ W0a + b0, feature-major, stored f32r in DRAM between evals
    p0_d = nc.dram_tensor("p0", [MC, 128, B], F32R)

    with tile.TileContext(nc) as tc:
        with (
            tc.tile_pool(name="const", bufs=1) as const,
            tc.tile_pool(name="acts", bufs=3) as acts,
            tc.tile_pool(name="tmp", bufs=5) as tmp,
            tc.tile_pool(name="tmp2", bufs=3) as tmp2,
            tc.tile_pool(name="p0pool", bufs=2) as p0pool,
            tc.tile_pool(name="msqp", bufs=2) as msqp,
            tc.tile_pool(name="statf", bufs=3) as statf,
            tc.tile_pool(name="statr", bufs=2) as statr,
            tc.tile_pool(name="abc", bufs=2) as abc,
            tc.tile_pool(name="stg", bufs=2) as stg,
            tc.tile_pool(name="outp", bufs=2) as outp,
            tc.tile_pool(name="pm", bufs=4, space="PSUM") as pm,
            tc.tile_pool(name="pstat", bufs=2, space="PSUM") as pstat,
            tc.tile_pool(name="pstatq", bufs=1, space="PSUM") as pstatq,
            tc.tile_pool(name="pb", bufs=1, space="PSUM") as pb,
        ):
            # ---------------- constants -----------------------------------
            ident = const.tile([128, 128], F32)
            make_identity(nc, ident[:])
            ones_f = const.tile([128, 1], F32)
            nc.vector.memset(ones_f[:], 1.0)
            ones = const.tile([128, 1], F32R)
            nc.scalar.copy(ones[:], ones_f[:])
            onesrow_f = const.tile([1, 128], F32)
            nc.vector.memset(onesrow_f[:], 1.0)
            onesrow = const.tile([1, 128], F32R)
            nc.scalar.copy(onesrow[:], onesrow_f[:])

            w0zx = const.tile([128, H], F32R)
            nc.gpsimd.dma_start(w0zx[:], w0zx_e.ap())
            w1p = const.tile([128, MC, H], F32R)
            nc.gpsimd.dma_start(
                w1p[:], w1p_e.ap().rearrange("(ko ki) m -> ki ko m", ki=128)
            )
            w2p = const.tile([128, MC, ACT_DIM], F32R)
            nc.gpsimd.dma_start(
                w2p[:], w2p_e.ap().rearrange("(ko ki) m -> ki ko m", ki=128)
            )
            negc1 = const.tile([1, H], F32R)
            nc.gpsimd.dma_start(negc1[:], negc1_e.ap())
            negc2 = const.tile([1, ACT_DIM], F32R)
            nc.gpsimd.dma_start(negc2[:], negc2_e.ap())
            b0c = const.tile([128, MC], F32)
            nc.sync.dma_start(b0c[:], b0c_e.ap())
            d1c = const.tile([128, MC], F32)
            nc.sync.dma_start(d1c[:], d1c_e.ap())
            dtd2 = const.tile([ACT_DIM, 1], F32)
            nc.sync.dma_start(dtd2[:], dtd2_e.ap())
            stdv = const.tile([ACT_DIM, 1], F32)
            nc.sync.dma_start(stdv[:], std_e.ap())

            # persistent state (feature-major). zx ping-pong: evals 0,2 read
            # zxa; evals 1,3 read zxb (rows 0:64 z / z_pred, row 64 tau).
            zxa = const.tile([128, B], F32R)
            zxb = const.tile([128, B], F32R)
            z = const.tile([ACT_DIM, B], F32)
            k1dt = const.tile([ACT_DIM, B], F32)
            tausrc = const.tile([1, NB], F32)

            # zero zx rows 64..127 (write via ACT so the f32r round is legal)
            zsrc = const.tile([64, NB], F32)
            nc.vector.memset(zsrc[:], 0.0)
            for s in range(n_bc):
                nc.scalar.copy(zxa[64:128, s * NB:(s + 1) * NB], zsrc[:])
                nc.scalar.copy(zxb[64:128, s * NB:(s + 1) * NB], zsrc[:])

            # ---------------- eps -> z0 (transpose + scale by std) ---------
            for bb in range(B // 128):
                stage = stg.tile([128, 128], F32, tag="stg")
                nc.sync.dma_start(stage[:, :ACT_DIM],
                                  eps_e[bb * 128:(bb + 1) * 128, :])
                pt = pb.tile([128, 512], F32, tag="pbt")
                nc.tensor.transpose(pt[:ACT_DIM, :128], stage[:, :ACT_DIM],
                                    ident[:])
                sl = slice(bb * 128, (bb + 1) * 128)
                nc.scalar.activation(z[:, sl], pt[:ACT_DIM, :128],
                                     AF.Identity, scale=stdv[:])
                nc.scalar.activation(zxa[0:ACT_DIM, sl], pt[:ACT_DIM, :128],
                                     AF.Identity, scale=stdv[:])

            # ---------------- GEMM0: P' = obs @ W0a + b0 -------------------
            w0a = acts.tile([128, 4, H], F32R, tag="acts")
            nc.gpsimd.dma_start(
                w0a[:], w0a_e.ap().rearrange("(ko ki) m -> ki ko m", ki=128)
            )
            for bc in range(n_bc):
                obst = acts.tile([128, 4, NB], F32R, tag="acts")
                for fb in range(4):
                    for sub in range(4):
                        stage = stg.tile([128, 128], F32, tag="stg")
                        nc.sync.dma_start(
                            stage[:],
                            obs_e[(bc * 4 + sub) * 128:(bc * 4 + sub + 1) * 128,
                                  fb * 128:(fb + 1) * 128],
                        )
                        pt = pb.tile([128, 512], F32, tag="pbt")
                        nc.tensor.transpose(pt[:, :128], stage[:], ident[:])
                        nc.scalar.copy(
                            obst[:, fb, sub * 128:(sub + 1) * 128], pt[:, :128]
                        )
                for mc in range(MC):
                    pp = pm.tile([128, NB], F32, tag="pm")
                    for fb in range(4):
                        nc.tensor.matmul(
                            pp[:], w0a[:, fb, mc * 128:(mc + 1) * 128],
                            obst[:, fb, :], start=(fb == 0), stop=(fb == 3),
                        )
                    dr = p0pool.tile([128, NB], F32R, tag="p0")
                    nc.scalar.activation(dr[:], pp[:], AF.Identity,
                                         bias=b0c[:, mc:mc + 1])
                    nc.sync.dma_start(p0_d[mc, :, bc * NB:(bc + 1) * NB], dr[:])

            # ---------------- helper: LN stats chain -----------------------
            def stats_chain(sps, spq):
                """sps/spq: psum [1,NB] (S and Q). Returns (mu f32r,
                a f32r) with mu = S/F, a = 1/sqrt(Q/F - mu^2 + eps)."""
                mu = statr.tile([1, NB], F32R, tag="statr")
                nc.vector.tensor_scalar_mul(mu[:], sps[:], 1.0 / H)
                musq = statf.tile([1, NB], F32, tag="statf")
                nc.vector.tensor_mul(musq[:], mu[:], mu[:])
                qfe = statf.tile([1, NB], F32, tag="statf")
                nc.vector.tensor_scalar(qfe[:], spq[:], 1.0 / H, LN_EPS,
                                        ALU.mult, ALU.add)
                var = statf.tile([1, NB], F32, tag="statf")
                nc.vector.tensor_tensor(var[:], qfe[:], musq[:], ALU.subtract)
                lv = statf.tile([1, NB], F32, tag="statf")
                nc.scalar.activation(lv[:], var[:], AF.Ln)
                a = statr.tile([1, NB], F32R, tag="statr")
                nc.scalar.activation(a[:], lv[:], AF.Exp, scale=-0.5)
                return mu, a

            def bcast(a_row):
                """[1,NB] f32r -> [128,NB] f32 via K=1 matmul + DVE copy."""
                pbt = pb.tile([128, 512], F32, tag="pbt")
                nc.tensor.matmul(pbt[:, :NB], onesrow[:], a_row[:],
                                 start=True, stop=True)
                ab = abc.tile([128, NB], F32, tag="abc")
                nc.vector.tensor_copy(ab[:], pbt[:, :NB])
                return ab

            # ---------------- the 4 velocity evals -------------------------
            for e in range(4):
                tau = TAUS[e]
                is_k1 = (e % 2 == 0)
                zxr = zxa if e % 2 == 0 else zxb   # tile read by this eval
                zxw = zxb if e % 2 == 0 else zxa   # tile written (z_pred/z)
                # tau row of the tile this eval reads
                nc.vector.memset(tausrc[:], tau)
                for s in range(n_bc):
                    nc.scalar.copy(zxr[64:65, s * NB:(s + 1) * NB], tausrc[:])

                for bc in range(n_bc):
                    bsl = slice(bc * NB, (bc + 1) * NB)
                    # ---- L0 ----
                    m0t = acts.tile([128, MC, NB], F32R, tag="acts")
                    sps = pstat.tile([1, NB], F32, tag="sum")
                    spq = pstatq.tile([1, NB], F32, tag="ssq")
                    for pr in range(MC // 2):
                        up = tmp2.tile([128, 2, NB], F32, tag="tmp2")
                        y0s = []
                        for j in range(2):
                            mc = pr * 2 + j
                            pp = pm.tile([128, NB], F32, tag="pm")
                            nc.tensor.matmul(
                                pp[:], w0zx[:, mc * 128:(mc + 1) * 128],
                                zxr[:, bsl], start=True, stop=True)
                            p0t = p0pool.tile([128, NB], F32, tag="p0")
                            nc.sync.dma_start(p0t[:],
                                              p0_d[mc, :, bsl].bitcast(F32))
                            y0t = tmp.tile([128, NB], F32, tag="tmp")
                            nc.vector.tensor_tensor(y0t[:], pp[:], p0t[:],
                                                    ALU.add)
                            nc.scalar.activation(up[:, j, :], y0t[:], AF.Exp)
                            y0s.append(y0t)
                        nc.scalar.activation(up[:], up[:], AF.Square, bias=1.0)
                        nc.scalar.activation(up[:], up[:], AF.Ln, bias=1.0)
                        nc.scalar.activation(up[:], up[:], AF.Exp, scale=-1.0)
                        nc.gpsimd.tensor_scalar(up[:], up[:], -2.0, 1.0,
                                                ALU.mult, ALU.add)
                        for j in range(2):
                            mc = pr * 2 + j
                            nc.vector.tensor_mul(m0t[:, mc, :], y0s[j][:],
                                                 up[:, j, :])
                            ms = msqp.tile([128, NB], F32R, tag="msq")
                            nc.gpsimd.tensor_tensor(ms[:], m0t[:, mc, :],
                                                    m0t[:, mc, :], ALU.mult)
                            nc.tensor.matmul(sps[:], ones[:],
                                             m0t[:, mc, :], start=(mc == 0),
                                             stop=(mc == MC - 1))
                            nc.tensor.matmul(spq[:], ones[:], ms[:],
                                             start=(mc == 0),
                                             stop=(mc == MC - 1))
                    mu0, a0 = stats_chain(sps, spq)
                    a0b = bcast(a0)
                    # ---- L1 ----
                    m1t = acts.tile([128, MC, NB], F32R, tag="acts")
                    sps1 = pstat.tile([1, NB], F32, tag="sum")
                    spq1 = pstatq.tile([1, NB], F32, tag="ssq")
                    for pr in range(MC // 2):
                        up = tmp2.tile([128, 2, NB], F32, tag="tmp2")
                        y1s = []
                        for j in range(2):
                            mc = pr * 2 + j
                            pp = pm.tile([128, NB], F32, tag="pm")
                            for kc in range(MC):
                                nc.tensor.matmul(
                                    pp[:], w1p[:, kc, mc * 128:(mc + 1) * 128],
                                    m0t[:, kc, :], start=(kc == 0), stop=False,
                                )
                            nc.tensor.matmul(
                                pp[:], negc1[:, mc * 128:(mc + 1) * 128],
                                mu0[:], start=False, stop=True)
                            t = tmp.tile([128, NB], F32, tag="tmp")
                            nc.vector.tensor_mul(t[:], pp[:], a0b[:])
                            nc.vector.tensor_scalar_add(t[:], t[:],
                                                        d1c[:, mc:mc + 1])
                            nc.scalar.activation(up[:, j, :], t[:], AF.Exp)
                            y1s.append(t)
                        nc.scalar.activation(up[:], up[:], AF.Square, bias=1.0)
                        nc.scalar.activation(up[:], up[:], AF.Ln, bias=1.0)
                        nc.scalar.activation(up[:], up[:], AF.Exp, scale=-1.0)
                        nc.gpsimd.tensor_scalar(up[:], up[:], -2.0, 1.0,
                                                ALU.mult, ALU.add)
                        for j in range(2):
                            mc = pr * 2 + j
                            nc.vector.tensor_mul(m1t[:, mc, :], y1s[j][:],
                                                 up[:, j, :])
                            ms = msqp.tile([128, NB], F32R, tag="msq")
                            nc.vector.tensor_mul(ms[:], m1t[:, mc, :],
                                                 m1t[:, mc, :])
                            nc.tensor.matmul(sps1[:], ones[:],
                                             m1t[:, mc, :], start=(mc == 0),
                                             stop=(mc == MC - 1))
                            nc.tensor.matmul(spq1[:], ones[:], ms[:],
                                             start=(mc == 0),
                                             stop=(mc == MC - 1))
                    mu1, a1 = stats_chain(sps1, spq1)
                    a1b = bcast(a1)
                    # ---- L2 (output head) ----
                    pv = pm.tile([128, NB], F32, tag="pm")
                    for kc in range(MC):
                        nc.tensor.matmul(pv[:ACT_DIM, :], w2p[:, kc, :],
                                         m1t[:, kc, :], start=(kc == 0),
                                         stop=False)
                    nc.tensor.matmul(pv[:ACT_DIM, :], negc2[:], mu1[:],
                                     start=False, stop=True)
                    t2 = tmp.tile([128, NB], F32, tag="tmp")
                    nc.vector.tensor_mul(t2[:ACT_DIM], pv[:ACT_DIM, :],
                                         a1b[:ACT_DIM])
                    # dk = dt*(v + d2) = dt*t2 + dt*d2
                    if is_k1:
                        nc.scalar.activation(k1dt[:, bsl], t2[:ACT_DIM],
                                             AF.Identity, bias=dtd2[:],
                                             scale=DT)
                        # z_pred into the other zx tile's rows 0:64
                        nc.vector.tensor_tensor(zxw[0:ACT_DIM, bsl], z[:, bsl],
                                                k1dt[:, bsl], ALU.add)
                    else:
                        dk = tmp.tile([128, NB], F32, tag="tmp")
                        nc.scalar.activation(dk[:ACT_DIM], t2[:ACT_DIM],
                                             AF.Identity, bias=dtd2[:],
                                             scale=DT)
                        s = tmp.tile([128, NB], F32, tag="tmp")
                        nc.vector.tensor_tensor(s[:ACT_DIM], k1dt[:, bsl],
                                                dk[:ACT_DIM], ALU.add)
                        h = tmp.tile([128, NB], F32, tag="tmp")
                        nc.vector.tensor_scalar_mul(h[:ACT_DIM], s[:ACT_DIM],
                                                    0.5)
                        nc.vector.tensor_tensor(z[:, bsl], z[:, bsl],
                                                h[:ACT_DIM], ALU.add)
                        if e == 1:
                            nc.scalar.copy(zxw[0:ACT_DIM, bsl], z[:, bsl])

            # ---------------- output: z^T -> out [B, 64] -------------------
            for bb in range(B // 128):
                pt = pb.tile([128, 512], F32, tag="pbt")
                nc.tensor.transpose(pt[:, :ACT_DIM],
                                    z[:, bb * 128:(bb + 1) * 128],
                                    ident[:ACT_DIM, :ACT_DIM])
                ot = outp.tile([128, ACT_DIM], F32, tag="out")
                nc.scalar.copy(ot[:], pt[:, :ACT_DIM])
                nc.sync.dma_start(out_e[bb * 128:(bb + 1) * 128, :], ot[:])

    _split_multi_waits(nc)
    return nc


# ---------------------------------------------------------------------------


def _host_params(inputs):
    obs = np.asarray(inputs["obs"], dtype=np.float32)
    eps = np.asarray(inputs["eps"], dtype=np.float32)
    logstd = np.asarray(inputs["logstd"], dtype=np.float32)
    W0 = np.asarray(inputs["W0"], dtype=np.float32)
    b0 = np.asarray(inputs["b0"], dtype=np.float32)
    g0 = np.asarray(inputs["ln0_g"], dtype=np.float32)
    be0 = np.asarray(inputs["ln0_b"], dtype=np.float32)
    W1 = np.asarray(inputs["W1"], dtype=np.float32)
    b1 = np.asarray(inputs["b1"], dtype=np.float32)
    g1 = np.asarray(inputs["ln1_g"], dtype=np.float32)
    be1 = np.asarray(inputs["ln1_b"], dtype=np.float32)
    W2 = np.asarray(inputs["W2"], dtype=np.float32)
    b2 = np.asarray(inputs["b2"], dtype=np.float32)

    std = np.exp(np.clip(logstd, MIN_LOGSTD, None)).astype(np.float32)

    w0a = np.ascontiguousarray(W0[:OBS_DIM])                      # [512,1024]
    w0zx = np.zeros((128, H), dtype=np.float32)
    w0zx[:ACT_DIM] = W0[OBS_DIM:OBS_DIM + ACT_DIM]
    w0zx[ACT_DIM] = W0[OBS_DIM + ACT_DIM]                         # tau row
    b0c = np.ascontiguousarray(b0.reshape(MC, 128).T)             # [128,8]

    w1p = (g0[:, None] * W1).astype(np.float32)                   # [1024,1024]
    negc1 = np.ascontiguousarray(-w1p.sum(axis=0)[None, :])       # [1,1024]
    d1 = (be0 @ W1 + b1).astype(np.float32)
    d1c = np.ascontiguousarray(d1.reshape(MC, 128).T)             # [128,8]

    w2p = (g1[:, None] * W2).astype(np.float32)                   # [1024,64]
    negc2 = np.ascontiguousarray(-w2p.sum(axis=0)[None, :])       # [1,64]
    d2 = (be1 @ W2 + b2).astype(np.float32)
    dtd2 = np.ascontiguousarray((DT * d2)[:, None])               # [64,1]

    shared = {
        "w0a": w0a, "w0zx": w0zx, "b0c": b0c,
        "w1p": w1p, "negc1": negc1, "d1c": d1c,
        "w2p": w2p, "negc2": negc2, "dtd2": dtd2,
        "std": np.ascontiguousarray(std[:, None]),
    }
    return obs, eps, shared


_graph_cache = {}


def kernel(**inputs):
    obs, eps, shared = _host_params(inputs)
    B = obs.shape[0]
    assert B % N_CORES == 0
    bc_per = B // N_CORES
    assert bc_per % NB == 0
    n_bc = bc_per // NB

    if n_bc not in _graph_cache:
        _graph_cache[n_bc] = build_graph(n_bc)
    nc = _graph_cache[n_bc]

    in_maps = []
    for c in range(N_CORES):
        sl = slice(c * bc_per, (c + 1) * bc_per)
        m = {"obs": np.ascontiguousarray(obs[sl]),
             "eps": np.ascontiguousarray(eps[sl])}
        m.update(shared)
        in_maps.append(m)

    res = run_bass_kernel_spmd(nc, in_maps, core_ids=list(range(N_CORES)))
    out = np.concatenate([res.results[c]["out"] for c in range(N_CORES)],
                         axis=0)
    return out.astype(np.float32)
